# revision 2
# baseline (speedup 1.0000x reference)
"""NT-Xent (SimCLR contrastive) loss on Trainium2, sharded across 8 NeuronCores.

Sharding: each core computes a [512, 4096] row-slice of the similarity matrix.
The host ships each core a ROTATED copy of z (rows rolled by -512*core) cast to
bf16 - pure indexing/dtype prep, no host arithmetic. Rotation makes the device
program core-uniform: own rows are always rotated rows 0..511, the positive
partners always rotated rows 2048..2559. Row-sums of softmax are invariant to
the column permutation the rotation induces.

Device pipeline (per core, SPMD, pipelined per 1024-column block; block 0 is
processed in two 512-wide chunks to shorten the pipeline fill):
  - column-major z^T tiles materialized by XBAR transpose-DMAs straight from
    the row-major DRAM tensor (zbT[p,k,c] = z[c, 128k+p])
  - column norms: squares (DVE bf16 2x) pair-added so the ones-matmul
    contraction is K=256 instead of K=512
  - rinv16 = exp(-0.5*ln(ssq) + ln(16*1.045)) (one shared ACT table set)
  - zn = z * rinv16 in bf16 (DVE 2x), cast to fp8e4 by SWDGE cast-DMA
    (off-engine; 1.045 pre-scale compensates HW truncate-toward-zero)
  - Gram slice: fp8 DoubleRow matmuls, exp row-sums fused into ScalarE's
    activation accumulator
  - diagonal: recomputed exactly from the quantized fp8 values (elementwise
    square + gpsimd partition_all_reduce), subtracted before the final ln
  - positives: bf16 elementwise prod + partition_all_reduce + free-axis reduce
"""

import numpy as np

B = 2048
D = 512
N2 = 2 * B              # 4096 total rows
NCORES = 8
RPC = N2 // NCORES      # 512 rows per core
KT = D // 128           # 4 contraction tiles
BLK = 1024              # column-block size
NBLK = N2 // BLK        # 4 blocks
TEMP = 0.1
SCALE = 1.0 / TEMP      # 10.0
FP8_SCALE = 16.0        # zn stored as fp8(zn*16.72); effective x16 after trunc
TRUNC_COMP = 1.045      # SWDGE cast truncates toward zero; pre-scale by ~half
S16 = FP8_SCALE * TRUNC_COMP
LN_S16 = float(np.log(S16))
SC_DR = SCALE / (FP8_SCALE ** 2)        # exp scale for fp8 Gram psum
POS_MUL = -SCALE / (S16 * S16) / 128.0  # pos prods carry S16^2

_CACHE = {}


def _patch_act_tables(nc, mybir):
    """Make Ln and Exp resolve to the shared natural_log_exp_and_others set
    so the compiler emits one ACT table load instead of thrashing."""
    from concourse import hw_specs

    tables = hw_specs.get_activation_tables(nc.m.arch)
    keep = "natural_log_exp_and_others"
    if keep not in tables:
        return
    F = mybir.ActivationFunctionType
    if F.Exp not in tables[keep] or F.Ln not in tables[keep]:
        return
    for name, fns in tables.items():
        if name != keep:
            fns.discard(F.Exp)
            fns.discard(F.Ln)


def _build():
    from concourse import bass, bacc, tile, mybir, bass_isa

    nc = bacc.Bacc("TRN2", target_bir_lowering=False, debug=False,
                   num_devices=NCORES)
    bf16 = mybir.dt.bfloat16
    f32 = mybir.dt.float32
    f8 = mybir.dt.float8e4
    F = mybir.ActivationFunctionType
    A = mybir.AluOpType
    AX = mybir.AxisListType
    DR = mybir.MatmulPerfMode.DoubleRow
    PSUM = bass.MemorySpace.PSUM

    z = nc.dram_tensor("z", [N2, D], bf16, kind="ExternalInput").ap()
    out = nc.dram_tensor("out", [1, 1], f32, kind="ExternalOutput").ap()

    with tile.TileContext(nc) as tc:
        with (
            tc.tile_pool(name="sb", bufs=1) as sb,
            tc.tile_pool(name="wrk", bufs=2) as wrk,
            tc.tile_pool(name="psG", bufs=2, space=PSUM) as psG,
            tc.tile_pool(name="psS", bufs=1, space=PSUM) as psS,
            tc.tile_pool(name="psM", bufs=1, space=PSUM) as psM,
        ):
            ones = sb.tile([128, 128], bf16, tag="ones")
            nc.vector.memset(ones[:], 1.0)
            bias16 = sb.tile([128, 1], f32, tag="b16")
            nc.vector.memset(bias16[:], LN_S16)
            bias10 = sb.tile([128, 1], f32, tag="b10")
            nc.vector.memset(bias10[:], SCALE)

            zbT = [sb.tile([128, KT, BLK], bf16, tag=f"zbT{b}", name=f"zbT{b}")
                   for b in range(NBLK)]
            znb0 = sb.tile([128, KT, BLK], bf16, tag="znb0")
            znb2 = sb.tile([128, KT, BLK], bf16, tag="znb2")
            zn8_0 = sb.tile([128, KT, BLK], f8, tag="zn8_0")
            rowp = sb.tile([128, 4, NBLK + 1], f32, tag="rowp")

            # PE p-state warm-up while the first DMAs stream in
            warm = psM.tile([128, 512], f32, tag="pd")
            for _ in range(8):
                nc.tensor.matmul(warm[:, 0:128], ones[:], ones[:],
                                 start=True, stop=True)

            # column-major tiles via XBAR transpose: zbT[p,k,c] = z[c,128k+p]
            nc.sync.dma_start(out=zbT[0][:, :, 0:512], in_=z[0:512, :],
                              transpose=True)
            nc.sync.dma_start(out=zbT[0][:, :, 512:1024], in_=z[512:1024, :],
                              transpose=True)
            for b in range(1, NBLK):
                nc.sync.dma_start(out=zbT[b][:], in_=z[b * BLK:(b + 1) * BLK, :],
                                  transpose=True)

            def norm(b, lo, hi, zn_dst, pssq):
                """ssq -> rinv16 -> zn (bf16) for columns [lo,hi) of block b."""
                W = hi - lo
                zb = zbT[b]
                sq = wrk.tile([128, KT, W], bf16, tag="sq", name="sq")
                nc.vector.tensor_tensor(sq[:], zb[:, :, lo:hi], zb[:, :, lo:hi],
                                        A.mult)
                sqp = wrk.tile([128, 2, W], bf16, tag="sqp", name="sqp")
                nc.vector.tensor_tensor(sqp[:], sq[:, 0:2, :], sq[:, 2:4, :],
                                        A.add)
                for g in range(2):
                    for j in range(W // 512):
                        nc.tensor.matmul(
                            pssq[:, lo + j * 512:lo + (j + 1) * 512],
                            ones[:], sqp[:, g, j * 512:(j + 1) * 512],
                            start=(g == 0), stop=(g == 1))
                lns = wrk.tile([128, W], f32, tag="lns", name="lns")
                nc.scalar.activation(lns[:], pssq[:, lo:hi], F.Ln)
                rin = wrk.tile([128, W], bf16, tag="rin", name="rin")
                nc.scalar.activation(rin[:], lns[:], F.Exp, scale=-0.5,
                                     bias=bias16[:])
                for k in range(KT):
                    nc.vector.tensor_tensor(zn_dst[:, k, lo:hi],
                                            zb[:, k, lo:hi], rin[:], A.mult)

            def gram_exp(rhs8, lo, hi, col):
                """DR Gram of own rows x cols [lo,hi), exp row-sums -> rowp."""
                W = hi - lo
                for m in range(4):
                    pm = psG.tile([128, BLK], f32, tag="gram", name="pm")
                    for g in range(2):
                        for j in range(W // 512):
                            nc.tensor.matmul(
                                pm[:, j * 512:(j + 1) * 512],
                                zn8_0[:, 2 * g:2 * g + 2, m * 128:(m + 1) * 128],
                                rhs8[:, 2 * g:2 * g + 2, lo + j * 512:lo + (j + 1) * 512],
                                start=(g == 0), stop=(g == 1), perf_mode=DR)
                    scr = wrk.tile([128, W], bf16, tag="scr", name="scr")
                    nc.scalar.activation(scr[:], pm[:, 0:W], F.Exp, scale=SC_DR,
                                         accum_out=rowp[:, m, col:col + 1])

            # ---- block 0 (own block) in two 512-wide chunks ----
            ps0 = psS.tile([128, BLK], f32, tag="ssq", name="ps0")
            norm(0, 0, 512, znb0, ps0)
            norm(0, 512, 1024, znb0, ps0)
            nc.gpsimd.dma_start(out=zn8_0[:, :, 0:512], in_=znb0[:, :, 0:512])
            nc.gpsimd.dma_start(out=zn8_0[:, :, 512:1024],
                                in_=znb0[:, :, 512:1024])

            # block 1 norm early so its ACT work fills gaps before the exps
            ps1 = psS.tile([128, BLK], f32, tag="ssq", name="ps1")
            znb1 = wrk.tile([128, KT, BLK], bf16, tag="znt", name="znb1")
            norm(1, 0, 1024, znb1, ps1)
            zn8_1 = wrk.tile([128, KT, BLK], f8, tag="zn8t", name="zn8_1")
            nc.gpsimd.dma_start(out=zn8_1[:], in_=znb1[:])

            gram_exp(zn8_0, 0, 512, 0)

            # diag: exact self-dot of the quantized own columns (rot cols 0:512)
            prodd = wrk.tile([128, KT, 512], bf16, tag="prodd", name="prodd")
            nc.vector.tensor_tensor(prodd[:], zn8_0[:, :, 0:512],
                                    zn8_0[:, :, 0:512], A.mult)
            dacc1 = wrk.tile([128, 2, 512], f32, tag="dacc1", name="dacc1")
            nc.vector.tensor_tensor(dacc1[:], prodd[:, 0:2, :], prodd[:, 2:4, :],
                                    A.add)
            dacc2 = wrk.tile([128, 512], f32, tag="dacc2", name="dacc2")
            nc.vector.tensor_tensor(dacc2[:], dacc1[:, 0, :], dacc1[:, 1, :],
                                    A.add)
            dg_bc = sb.tile([128, 512], f32, tag="dgbc")
            nc.gpsimd.partition_all_reduce(dg_bc[:], dacc2[:], channels=128,
                                           reduce_op=bass_isa.ReduceOp.add)
            diag_row = sb.tile([1, 512], bf16, tag="diagrow")
            nc.vector.tensor_scalar_add(diag_row[:], dg_bc[0:1, :],
                                        -FP8_SCALE ** 2)

            gram_exp(zn8_0, 512, 1024, 1)

            # block 2 (partner block) norm; znb2 kept for the positives
            ps2 = psS.tile([128, BLK], f32, tag="ssq", name="ps2")
            norm(2, 0, 1024, znb2, ps2)
            zn8_2 = wrk.tile([128, KT, BLK], f8, tag="zn8t", name="zn8_2")
            nc.gpsimd.dma_start(out=zn8_2[:], in_=znb2[:])

            gram_exp(zn8_1, 0, 1024, 2)

            # positives: pos_r = zn_r . zn_{r+2048} in bf16 (rot partner rows
            # 2048..2559 = block-2 cols 0:512, same partition alignment)
            prodp = wrk.tile([128, KT, 512], bf16, tag="prodp", name="prodp")
            nc.vector.tensor_tensor(prodp[:], znb0[:, :, 0:512],
                                    znb2[:, :, 0:512], A.mult)
            pacc1 = wrk.tile([128, 2, 512], f32, tag="pacc1", name="pacc1")
            nc.vector.tensor_tensor(pacc1[:], prodp[:, 0:2, :], prodp[:, 2:4, :],
                                    A.add)
            pacc2 = wrk.tile([128, 512], f32, tag="pacc2", name="pacc2")
            nc.vector.tensor_tensor(pacc2[:], pacc1[:, 0, :], pacc1[:, 1, :],
                                    A.add)
            pos_bc = sb.tile([128, 512], f32, tag="posbc")
            nc.gpsimd.partition_all_reduce(pos_bc[:], pacc2[:], channels=128,
                                           reduce_op=bass_isa.ReduceOp.add)
            pos_red = sb.tile([128, 1], f32, tag="posr")
            nc.vector.tensor_reduce(pos_red[:], pos_bc[:], AX.X, A.add)

            # block 3 norm
            ps3 = psS.tile([128, BLK], f32, tag="ssq", name="ps3")
            znb3 = wrk.tile([128, KT, BLK], bf16, tag="znt", name="znb3")
            norm(3, 0, 1024, znb3, ps3)
            zn8_3 = wrk.tile([128, KT, BLK], f8, tag="zn8t", name="zn8_3")
            nc.gpsimd.dma_start(out=zn8_3[:], in_=znb3[:])

            gram_exp(zn8_2, 0, 1024, 3)

            # diag -> partition layout via K=1 outer-product matmuls
            dt = psM.tile([128, 512], f32, tag="pd")
            for m in range(4):
                nc.tensor.matmul(dt[:, m * 128:(m + 1) * 128],
                                 diag_row[0:1, m * 128:(m + 1) * 128],
                                 ones[0:1, 0:128], start=True, stop=True)
            diag_part = sb.tile([128, 4], f32, tag="diagp")
            for m in range(4):
                nc.vector.tensor_copy(diag_part[:, m:m + 1],
                                      dt[:, m * 128:m * 128 + 1])
            dexp = sb.tile([128, 4], f32, tag="dexp")
            nc.scalar.activation(dexp[:], diag_part[:], F.Exp, scale=SC_DR,
                                 bias=bias10[:])

            gram_exp(zn8_3, 0, 1024, 4)

            # ---- finale: partial = sum_r ln(Z_r) - 10 * sum_r pos_r ----
            zsum = sb.tile([128, 4], f32, tag="zsum")
            for m in range(4):
                nc.vector.tensor_reduce(zsum[:, m:m + 1], rowp[:, m, :],
                                        AX.X, A.add)
            zarg = sb.tile([128, 4], f32, tag="zarg")
            nc.vector.tensor_tensor(zarg[:], zsum[:], dexp[:], A.subtract)
            logz = sb.tile([128, 5], f32, tag="logz")
            nc.scalar.activation(logz[:, 0:4], zarg[:], F.Ln)
            nc.vector.tensor_scalar_mul(logz[:, 4:5], pos_red[:], POS_MUL)
            red1 = sb.tile([128, 1], f32, tag="red1")
            nc.vector.tensor_reduce(red1[:], logz[:], AX.X, A.add)
            fin = sb.tile([1, 1], f32, tag="fin")
            nc.gpsimd.tensor_reduce(fin[:], red1[:], AX.C, A.add)
            nc.sync.dma_start(out=out, in_=fin[:])

    _patch_act_tables(nc, mybir)
    nc.compile()
    return nc


def _get_nc():
    if "nc" not in _CACHE:
        _CACHE["nc"] = _build()
    return _CACHE["nc"]


def _in_maps(z_i, z_j):
    import ml_dtypes

    zf = np.concatenate(
        [np.asarray(z_i, np.float32), np.asarray(z_j, np.float32)], axis=0)
    zb = zf.astype(ml_dtypes.bfloat16)
    maps = []
    for c in range(NCORES):
        maps.append({"z": np.ascontiguousarray(np.roll(zb, -c * RPC, axis=0))})
    return maps


def _run(z_i, z_j, trace=False):
    from concourse.bass_utils import run_bass_kernel_spmd

    nc = _get_nc()
    return run_bass_kernel_spmd(nc, _in_maps(z_i, z_j), list(range(NCORES)),
                                trace=trace)


def kernel(z_i, z_j):
    res = _run(z_i, z_j, trace=False)
    total = sum(float(r["out"][0, 0]) for r in res.results)
    return np.float32(total / N2)


# revision 8
# speedup vs baseline: 1.0601x; 1.0601x over previous
"""NT-Xent (SimCLR contrastive) loss on Trainium2, sharded across 8 NeuronCores.

Sharding: each core computes a [512, 4096] row-slice of the similarity matrix.
The host ships each core a ROTATED copy of z (rows rolled by -512*core) cast to
bf16 - pure indexing/dtype prep, no host arithmetic. Rotation makes the device
program core-uniform: own rows are always rotated rows 0..511, the positive
partners always rotated rows 2048..2559. Row-sums of softmax are invariant to
the column permutation the rotation induces.

Device pipeline (per core, SPMD, pipelined per 1024-column block; block 0 is
processed in two 512-wide chunks to shorten the pipeline fill):
  - column-major z^T tiles materialized by XBAR transpose-DMAs straight from
    the row-major DRAM tensor (zbT[p,k,c] = z[c, 128k+p])
  - column norms: squares (DVE bf16 2x) pair-added so the ones-matmul
    contraction is K=256 instead of K=512
  - rinv16 = exp(-0.5*ln(ssq) + ln(16*1.045)) (one shared ACT table set)
  - zn = z * rinv16 in bf16 (DVE 2x), cast to fp8e4 by SWDGE cast-DMA
    (off-engine; 1.045 pre-scale compensates HW truncate-toward-zero)
  - Gram slice: fp8 DoubleRow matmuls, exp row-sums fused into ScalarE's
    activation accumulator
  - diagonal: recomputed exactly from the quantized fp8 values (elementwise
    square + gpsimd partition_all_reduce), subtracted before the final ln
  - positives: bf16 elementwise prod + partition_all_reduce + free-axis reduce
"""

import numpy as np

B = 2048
D = 512
N2 = 2 * B              # 4096 total rows
NCORES = 8
RPC = N2 // NCORES      # 512 rows per core
KT = D // 128           # 4 contraction tiles
BLK = 1024              # column-block size
NBLK = N2 // BLK        # 4 blocks
TEMP = 0.1
SCALE = 1.0 / TEMP      # 10.0
FP8_SCALE = 16.0        # zn stored as fp8(zn*16.72); effective x16 after trunc
TRUNC_COMP = 1.045      # SWDGE cast truncates toward zero; pre-scale by ~half
S16 = FP8_SCALE * TRUNC_COMP
LN_S16 = float(np.log(S16))
SC_DR = SCALE / (FP8_SCALE ** 2)        # exp scale for fp8 Gram psum
POS_MUL = -SCALE / (S16 * S16) / 128.0  # pos prods carry S16^2

_CACHE = {}


def _patch_act_tables(nc, mybir):
    """Make Ln and Exp resolve to the shared natural_log_exp_and_others set
    so the compiler emits one ACT table load instead of thrashing."""
    from concourse import hw_specs

    tables = hw_specs.get_activation_tables(nc.m.arch)
    keep = "natural_log_exp_and_others"
    if keep not in tables:
        return
    F = mybir.ActivationFunctionType
    if F.Exp not in tables[keep] or F.Ln not in tables[keep]:
        return
    for name, fns in tables.items():
        if name != keep:
            fns.discard(F.Exp)
            fns.discard(F.Ln)


def _build():
    from concourse import bass, bacc, tile, mybir, bass_isa

    nc = bacc.Bacc("TRN2", target_bir_lowering=False, debug=False,
                   num_devices=NCORES)
    bf16 = mybir.dt.bfloat16
    f32 = mybir.dt.float32
    f8 = mybir.dt.float8e4
    F = mybir.ActivationFunctionType
    A = mybir.AluOpType
    AX = mybir.AxisListType
    DR = mybir.MatmulPerfMode.DoubleRow
    PSUM = bass.MemorySpace.PSUM

    zt = nc.dram_tensor("zt", [D, N2], bf16, kind="ExternalInput").ap()
    out = nc.dram_tensor("out", [1, 1], f32, kind="ExternalOutput").ap()
    # [p, k, c] view: element (p, k, c) = zt[k*128 + p, c]
    zt_r = zt.rearrange("(k p) c -> p k c", k=KT)

    with tile.TileContext(nc) as tc:
        with (
            tc.tile_pool(name="sb", bufs=1) as sb,
            tc.tile_pool(name="wrk", bufs=2) as wrk,
            tc.tile_pool(name="psG", bufs=2, space=PSUM) as psG,
            tc.tile_pool(name="psS", bufs=1, space=PSUM) as psS,
            tc.tile_pool(name="psM", bufs=1, space=PSUM) as psM,
        ):
            ones = sb.tile([128, 128], bf16, tag="ones")
            nc.vector.memset(ones[:], 1.0)
            bias16 = sb.tile([128, 1], f32, tag="b16")
            nc.vector.memset(bias16[:], LN_S16)
            bias10 = sb.tile([128, 1], f32, tag="b10")
            nc.vector.memset(bias10[:], SCALE)

            zbT = [sb.tile([128, KT, BLK], bf16, tag=f"zbT{b}", name=f"zbT{b}")
                   for b in range(NBLK)]
            znb0 = sb.tile([128, KT, BLK], bf16, tag="znb0")
            znb2 = sb.tile([128, KT, BLK], bf16, tag="znb2")
            zn8_0 = sb.tile([128, KT, BLK], f8, tag="zn8_0")
            rowp = sb.tile([128, 4, NBLK + 1], f32, tag="rowp")

            # PE p-state warm-up while the first DMAs stream in
            warm = psM.tile([128, 512], f32, tag="pd")
            for _ in range(8):
                nc.tensor.matmul(warm[:, 0:128], ones[:], ones[:],
                                 start=True, stop=True)

            # column-major tiles, spread over three DMA queues (SP, ACT
            # HWDGE, Pool SWDGE) so all input lands within ~5us
            nc.sync.dma_start(out=zbT[0][:, :, 0:512], in_=zt_r[:, :, 0:512])
            nc.scalar.dma_start(out=zbT[1][:], in_=zt_r[:, :, BLK:2 * BLK])
            nc.gpsimd.dma_start(out=zbT[3][:], in_=zt_r[:, :, 3 * BLK:4 * BLK])
            nc.sync.dma_start(out=zbT[0][:, :, 512:1024],
                              in_=zt_r[:, :, 512:1024])
            nc.sync.dma_start(out=zbT[2][:], in_=zt_r[:, :, 2 * BLK:3 * BLK])

            # preload the Ln/Exp ACT table during the DMA window so the
            # 1.3us table load isn't on the first Ln's critical path
            junk1 = sb.tile([128, 1], f32, tag="junk1")
            nc.scalar.activation(junk1[:], bias10[:], F.Exp)

            def norm(b, lo, hi, zn_dst, pssq):
                """ssq -> rinv16 -> zn (bf16) for columns [lo,hi) of block b."""
                W = hi - lo
                zb = zbT[b]
                sq = wrk.tile([128, KT, W], bf16, tag="sq", name="sq")
                nc.vector.tensor_tensor(sq[:], zb[:, :, lo:hi], zb[:, :, lo:hi],
                                        A.mult)
                sqp = wrk.tile([128, 2, W], bf16, tag="sqp", name="sqp")
                nc.vector.tensor_tensor(sqp[:], sq[:, 0:2, :], sq[:, 2:4, :],
                                        A.add)
                for g in range(2):
                    for j in range(W // 512):
                        nc.tensor.matmul(
                            pssq[:, lo + j * 512:lo + (j + 1) * 512],
                            ones[:], sqp[:, g, j * 512:(j + 1) * 512],
                            start=(g == 0), stop=(g == 1))
                lns = wrk.tile([128, W], f32, tag="lns", name="lns")
                nc.scalar.activation(lns[:], pssq[:, lo:hi], F.Ln)
                rin = wrk.tile([128, W], bf16, tag="rin", name="rin")
                nc.scalar.activation(rin[:], lns[:], F.Exp, scale=-0.5,
                                     bias=bias16[:])
                for k in range(KT):
                    nc.vector.tensor_tensor(zn_dst[:, k, lo:hi],
                                            zb[:, k, lo:hi], rin[:], A.mult)

            def gram_exp(rhs8, lo, hi, col):
                """DR Gram of own rows x cols [lo,hi), exp row-sums -> rowp."""
                W = hi - lo
                for m in range(4):
                    pm = psG.tile([128, BLK], f32, tag="gram", name="pm")
                    for g in range(2):
                        for j in range(W // 512):
                            nc.tensor.matmul(
                                pm[:, j * 512:(j + 1) * 512],
                                zn8_0[:, 2 * g:2 * g + 2, m * 128:(m + 1) * 128],
                                rhs8[:, 2 * g:2 * g + 2, lo + j * 512:lo + (j + 1) * 512],
                                start=(g == 0), stop=(g == 1), perf_mode=DR)
                    scr = wrk.tile([128, W], bf16, tag="scr", name="scr")
                    nc.scalar.activation(scr[:], pm[:, 0:W], F.Exp, scale=SC_DR,
                                         accum_out=rowp[:, m, col:col + 1])

            # ---- block 0 (own block) in two 512-wide chunks ----
            ps0 = psS.tile([128, BLK], f32, tag="ssq", name="ps0")
            norm(0, 0, 512, znb0, ps0)
            norm(0, 512, 1024, znb0, ps0)
            nc.gpsimd.dma_start(out=zn8_0[:, :, 0:512], in_=znb0[:, :, 0:512])
            nc.gpsimd.dma_start(out=zn8_0[:, :, 512:1024],
                                in_=znb0[:, :, 512:1024])

            # block 1 norm early so its ACT work fills gaps before the exps
            ps1 = psS.tile([128, BLK], f32, tag="ssq", name="ps1")
            znb1 = wrk.tile([128, KT, BLK], bf16, tag="znt", name="znb1")
            norm(1, 0, 1024, znb1, ps1)
            zn8_1 = wrk.tile([128, KT, BLK], f8, tag="zn8t", name="zn8_1")
            nc.gpsimd.dma_start(out=zn8_1[:], in_=znb1[:])

            gram_exp(zn8_0, 0, 512, 0)

            # diag: exact self-dot of the quantized own columns (rot cols 0:512)
            prodd = wrk.tile([128, KT, 512], bf16, tag="prodd", name="prodd")
            nc.vector.tensor_tensor(prodd[:], zn8_0[:, :, 0:512],
                                    zn8_0[:, :, 0:512], A.mult)
            dacc1 = wrk.tile([128, 2, 512], f32, tag="dacc1", name="dacc1")
            nc.vector.tensor_tensor(dacc1[:], prodd[:, 0:2, :], prodd[:, 2:4, :],
                                    A.add)
            dacc2 = wrk.tile([128, 512], f32, tag="dacc2", name="dacc2")
            nc.vector.tensor_tensor(dacc2[:], dacc1[:, 0, :], dacc1[:, 1, :],
                                    A.add)

            gram_exp(zn8_0, 512, 1024, 1)

            # block 2 (partner block) norm; znb2 kept for the positives
            ps2 = psS.tile([128, BLK], f32, tag="ssq", name="ps2")
            norm(2, 0, 1024, znb2, ps2)
            zn8_2 = wrk.tile([128, KT, BLK], f8, tag="zn8t", name="zn8_2")
            nc.gpsimd.dma_start(out=zn8_2[:], in_=znb2[:])

            gram_exp(zn8_1, 0, 1024, 2)

            # positives: pos_r = zn_r . zn_{r+2048} in bf16 (rot partner rows
            # 2048..2559 = block-2 cols 0:512, same partition alignment)
            prodp = wrk.tile([128, KT, 512], bf16, tag="prodp", name="prodp")
            nc.vector.tensor_tensor(prodp[:], znb0[:, :, 0:512],
                                    znb2[:, :, 0:512], A.mult)
            pacc1 = wrk.tile([128, 2, 512], f32, tag="pacc1", name="pacc1")
            nc.vector.tensor_tensor(pacc1[:], prodp[:, 0:2, :], prodp[:, 2:4, :],
                                    A.add)
            pacc2 = wrk.tile([128, 512], f32, tag="pacc2", name="pacc2")
            nc.vector.tensor_tensor(pacc2[:], pacc1[:, 0, :], pacc1[:, 1, :],
                                    A.add)

            # block 3 norm
            ps3 = psS.tile([128, BLK], f32, tag="ssq", name="ps3")
            znb3 = wrk.tile([128, KT, BLK], bf16, tag="znt", name="znb3")
            norm(3, 0, 1024, znb3, ps3)
            zn8_3 = wrk.tile([128, KT, BLK], f8, tag="zn8t", name="zn8_3")
            nc.gpsimd.dma_start(out=zn8_3[:], in_=znb3[:])

            # all-reduces AFTER the last cast so they never block the fp8
            # cast pipeline on the single Pool SWDGE queue
            dg_bc = sb.tile([128, 512], f32, tag="dgbc")
            nc.gpsimd.partition_all_reduce(dg_bc[:], dacc2[:], channels=128,
                                           reduce_op=bass_isa.ReduceOp.add)
            diag_row = sb.tile([1, 512], bf16, tag="diagrow")
            nc.vector.tensor_scalar_add(diag_row[:], dg_bc[0:1, :],
                                        -FP8_SCALE ** 2)
            pos_bc = sb.tile([128, 512], f32, tag="posbc")
            nc.gpsimd.partition_all_reduce(pos_bc[:], pacc2[:], channels=128,
                                           reduce_op=bass_isa.ReduceOp.add)
            pos_red = sb.tile([128, 1], f32, tag="posr")
            nc.vector.tensor_reduce(pos_red[:], pos_bc[:], AX.X, A.add)

            gram_exp(zn8_2, 0, 1024, 3)

            # diag -> partition layout via K=1 outer-product matmuls
            dt = psM.tile([128, 512], f32, tag="pd")
            for m in range(4):
                nc.tensor.matmul(dt[:, m * 128:(m + 1) * 128],
                                 diag_row[0:1, m * 128:(m + 1) * 128],
                                 ones[0:1, 0:128], start=True, stop=True)
            diag_part = sb.tile([128, 4], f32, tag="diagp")
            for m in range(4):
                nc.vector.tensor_copy(diag_part[:, m:m + 1],
                                      dt[:, m * 128:m * 128 + 1])
            dexp = sb.tile([128, 4], f32, tag="dexp")
            nc.scalar.activation(dexp[:], diag_part[:], F.Exp, scale=SC_DR,
                                 bias=bias10[:])

            gram_exp(zn8_3, 0, 1024, 4)

            # ---- finale: partial = sum_r ln(Z_r) - 10 * sum_r pos_r ----
            zsum = sb.tile([128, 4], f32, tag="zsum")
            for m in range(4):
                nc.vector.tensor_reduce(zsum[:, m:m + 1], rowp[:, m, :],
                                        AX.X, A.add)
            zarg = sb.tile([128, 4], f32, tag="zarg")
            nc.vector.tensor_tensor(zarg[:], zsum[:], dexp[:], A.subtract)
            logz = sb.tile([128, 5], f32, tag="logz")
            nc.scalar.activation(logz[:, 0:4], zarg[:], F.Ln)
            nc.vector.tensor_scalar_mul(logz[:, 4:5], pos_red[:], POS_MUL)
            red1 = sb.tile([128, 1], f32, tag="red1")
            nc.vector.tensor_reduce(red1[:], logz[:], AX.X, A.add)
            fin = sb.tile([1, 1], f32, tag="fin")
            nc.gpsimd.tensor_reduce(fin[:], red1[:], AX.C, A.add)
            nc.sync.dma_start(out=out, in_=fin[:])

    _patch_act_tables(nc, mybir)
    nc.compile()
    return nc


def _get_nc():
    if "nc" not in _CACHE:
        _CACHE["nc"] = _build()
    return _CACHE["nc"]


def _in_maps(z_i, z_j):
    import ml_dtypes

    zf = np.concatenate(
        [np.asarray(z_i, np.float32), np.asarray(z_j, np.float32)], axis=0)
    zb = zf.astype(ml_dtypes.bfloat16)
    maps = []
    for c in range(NCORES):
        zr = np.roll(zb, -c * RPC, axis=0)
        maps.append({"zt": np.ascontiguousarray(zr.T)})
    return maps


def _run(z_i, z_j, trace=False):
    from concourse.bass_utils import run_bass_kernel_spmd

    nc = _get_nc()
    return run_bass_kernel_spmd(nc, _in_maps(z_i, z_j), list(range(NCORES)),
                                trace=trace)


def kernel(z_i, z_j):
    res = _run(z_i, z_j, trace=False)
    total = sum(float(r["out"][0, 0]) for r in res.results)
    return np.float32(total / N2)


# revision 10
# speedup vs baseline: 1.1493x; 1.0841x over previous
"""NT-Xent (SimCLR contrastive) loss on Trainium2, sharded across 8 NeuronCores.

Sharding: each core computes a [512, 4096] row-slice of the similarity matrix.
The host ships each core a ROTATED copy of z (rows rolled by -512*core) cast to
bf16 - pure indexing/dtype prep, no host arithmetic. Rotation makes the device
program core-uniform: own rows are always rotated rows 0..511, the positive
partners always rotated rows 2048..2559. Row-sums of softmax are invariant to
the column permutation the rotation induces.

Device pipeline (per core, SPMD, pipelined per 1024-column block; block 0 is
processed in two 512-wide chunks to shorten the pipeline fill):
  - column-major z^T tiles materialized by XBAR transpose-DMAs straight from
    the row-major DRAM tensor (zbT[p,k,c] = z[c, 128k+p])
  - column norms: squares (DVE bf16 2x) pair-added so the ones-matmul
    contraction is K=256 instead of K=512
  - rinv16 = exp(-0.5*ln(ssq) + ln(16*1.045)) (one shared ACT table set)
  - zn = z * rinv16 in bf16 (DVE 2x), cast to fp8e4 by SWDGE cast-DMA
    (off-engine; 1.045 pre-scale compensates HW truncate-toward-zero)
  - Gram slice: fp8 DoubleRow matmuls, exp row-sums fused into ScalarE's
    activation accumulator
  - diagonal: recomputed exactly from the quantized fp8 values (elementwise
    square + gpsimd partition_all_reduce), subtracted before the final ln
  - positives: bf16 elementwise prod + partition_all_reduce + free-axis reduce
"""

import numpy as np

B = 2048
D = 512
N2 = 2 * B              # 4096 total rows
NCORES = 8
RPC = N2 // NCORES      # 512 rows per core
KT = D // 128           # 4 contraction tiles
BLK = 1024              # column-block size
NBLK = N2 // BLK        # 4 blocks
TEMP = 0.1
SCALE = 1.0 / TEMP      # 10.0
FP8_SCALE = 16.0        # zn stored as fp8(zn*16.72); effective x16 after trunc
TRUNC_COMP = 1.045      # SWDGE cast truncates toward zero; pre-scale by ~half
S16 = FP8_SCALE * TRUNC_COMP
LN_S16 = float(np.log(S16))
SC_DR = SCALE / (FP8_SCALE ** 2)        # exp scale for fp8 Gram psum
POS_MUL = -SCALE / (S16 * S16) / 128.0  # pos prods carry S16^2

_CACHE = {}


def _patch_act_tables(nc, mybir):
    """Make Ln and Exp resolve to the shared natural_log_exp_and_others set
    so the compiler emits one ACT table load instead of thrashing."""
    from concourse import hw_specs

    tables = hw_specs.get_activation_tables(nc.m.arch)
    keep = "natural_log_exp_and_others"
    if keep not in tables:
        return
    F = mybir.ActivationFunctionType
    if F.Exp not in tables[keep] or F.Ln not in tables[keep]:
        return
    for name, fns in tables.items():
        if name != keep:
            fns.discard(F.Exp)
            fns.discard(F.Ln)


def _build():
    from concourse import bass, bacc, tile, mybir, bass_isa

    nc = bacc.Bacc("TRN2", target_bir_lowering=False, debug=False,
                   num_devices=NCORES)
    bf16 = mybir.dt.bfloat16
    f32 = mybir.dt.float32
    f8 = mybir.dt.float8e4
    F = mybir.ActivationFunctionType
    A = mybir.AluOpType
    AX = mybir.AxisListType
    DR = mybir.MatmulPerfMode.DoubleRow
    PSUM = bass.MemorySpace.PSUM

    zt = nc.dram_tensor("zt", [D, N2], bf16, kind="ExternalInput").ap()
    out = nc.dram_tensor("out", [1, 1], f32, kind="ExternalOutput").ap()
    # [p, k, c] view: element (p, k, c) = zt[k*128 + p, c]
    zt_r = zt.rearrange("(k p) c -> p k c", k=KT)

    with tile.TileContext(nc) as tc:
        with (
            tc.tile_pool(name="sb", bufs=1) as sb,
            tc.tile_pool(name="wrk", bufs=2) as wrk,
            tc.tile_pool(name="psG", bufs=2, space=PSUM) as psG,
            tc.tile_pool(name="psS", bufs=2, space=PSUM) as psS,
        ):
            ones = sb.tile([128, 128], bf16, tag="ones")
            nc.vector.memset(ones[:], 1.0)
            bias16 = sb.tile([128, 1], f32, tag="b16")
            nc.vector.memset(bias16[:], LN_S16)
            bias10 = sb.tile([128, 1], f32, tag="b10")
            nc.vector.memset(bias10[:], SCALE)

            zbT = [sb.tile([128, KT, BLK], bf16, tag=f"zbT{b}", name=f"zbT{b}")
                   for b in range(NBLK)]
            znb0 = sb.tile([128, KT, BLK], bf16, tag="znb0")
            znb2 = sb.tile([128, KT, BLK], bf16, tag="znb2")
            zn8_0 = sb.tile([128, KT, BLK], f8, tag="zn8_0")
            rowp = sb.tile([128, 4, NBLK + 1], f32, tag="rowp")

            # PE p-state warm-up while the first DMAs stream in
            warm = psG.tile([128, BLK], f32, tag="gram", name="warm")
            for _ in range(8):
                nc.tensor.matmul(warm[:, 0:128], ones[:], ones[:],
                                 start=True, stop=True)

            # column-major tiles, spread over three DMA queues (SP, ACT
            # HWDGE, Pool SWDGE) so all input lands within ~5us
            nc.sync.dma_start(out=zbT[0][:, :, 0:512], in_=zt_r[:, :, 0:512])
            nc.scalar.dma_start(out=zbT[1][:], in_=zt_r[:, :, BLK:2 * BLK])
            nc.gpsimd.dma_start(out=zbT[3][:], in_=zt_r[:, :, 3 * BLK:4 * BLK])
            nc.sync.dma_start(out=zbT[0][:, :, 512:1024],
                              in_=zt_r[:, :, 512:1024])
            nc.sync.dma_start(out=zbT[2][:], in_=zt_r[:, :, 2 * BLK:3 * BLK])

            # preload the Ln/Exp ACT table during the DMA window so the
            # 1.3us table load isn't on the first Ln's critical path
            junk1 = sb.tile([128, 1], f32, tag="junk1")
            nc.scalar.activation(junk1[:], bias10[:], F.Ln)

            # ---- split norm pipeline stages so each engine's in-order ----
            # ---- queue can be sequenced explicitly                    ----
            sqps = {}

            def norm_sq(b, lo, hi):
                """DVE: squares + pair-add over k (halves the ssq contraction)."""
                W = hi - lo
                sq = wrk.tile([128, KT, W], bf16, tag="sq", name="sq")
                nc.vector.tensor_tensor(sq[:], zbT[b][:, :, lo:hi],
                                        zbT[b][:, :, lo:hi], A.mult)
                sqp = wrk.tile([128, 2, W], bf16, tag="sqp", name="sqp")
                nc.vector.tensor_tensor(sqp[:], sq[:, 0:2, :], sq[:, 2:4, :],
                                        A.add)
                sqps[(b, lo)] = sqp

            def norm_ssq(b, lo, hi, pssq):
                """PE: ones-matmul partition reduction, K=2x128."""
                W = hi - lo
                sqp = sqps[(b, lo)]
                for g in range(2):
                    for j in range(W // 512):
                        nc.tensor.matmul(
                            pssq[:, lo + j * 512:lo + (j + 1) * 512],
                            ones[:], sqp[:, g, j * 512:(j + 1) * 512],
                            start=(g == 0), stop=(g == 1))

            def norm_act(b, lo, hi, pssq):
                """ACT: rinv16 = exp(-0.5*ln(ssq) + ln(16*1.045))."""
                W = hi - lo
                lns = wrk.tile([128, W], f32, tag="lns", name="lns")
                nc.scalar.activation(lns[:], pssq[:, lo:hi], F.Ln)
                rin = wrk.tile([128, W], bf16, tag="rin", name="rin")
                nc.scalar.activation(rin[:], lns[:], F.Exp, scale=-0.5,
                                     bias=bias16[:])
                return rin

            def norm_zn(b, lo, hi, zn_dst, rin):
                """DVE: zn = z * rinv16 (bf16, 2x mode)."""
                for k in range(KT):
                    nc.vector.tensor_tensor(zn_dst[:, k, lo:hi],
                                            zbT[b][:, k, lo:hi], rin[:], A.mult)

            def gram_exp(rhs8, lo, hi, col):
                """PE DR Gram of own rows x cols [lo,hi); ACT exp row-sums."""
                W = hi - lo
                for m in range(4):
                    pm = psG.tile([128, BLK], f32, tag="gram", name="pm")
                    for g in range(2):
                        for j in range(W // 512):
                            nc.tensor.matmul(
                                pm[:, j * 512:(j + 1) * 512],
                                zn8_0[:, 2 * g:2 * g + 2, m * 128:(m + 1) * 128],
                                rhs8[:, 2 * g:2 * g + 2, lo + j * 512:lo + (j + 1) * 512],
                                start=(g == 0), stop=(g == 1), perf_mode=DR)
                    scr = wrk.tile([128, W], bf16, tag="scr", name="scr")
                    nc.scalar.activation(scr[:], pm[:, 0:W], F.Exp, scale=SC_DR,
                                         accum_out=rowp[:, m, col:col + 1])

            # ---- DVE spine: all squares as early as their DMAs allow, ----
            # ---- zn multiplies interleaved as the rinvs arrive        ----
            ps0 = psS.tile([128, BLK], f32, tag="ssq", name="ps0")
            ps1 = psS.tile([128, BLK], f32, tag="ssq", name="ps1")
            ps2 = psS.tile([128, BLK], f32, tag="ssq", name="ps2")
            ps3 = psS.tile([128, BLK], f32, tag="ssq", name="ps3")
            znb1 = wrk.tile([128, KT, BLK], bf16, tag="znt", name="znb1")
            znb3 = wrk.tile([128, KT, BLK], bf16, tag="znt", name="znb3")
            zn8_1 = wrk.tile([128, KT, BLK], f8, tag="zn8t", name="zn8_1")
            zn8_2 = wrk.tile([128, KT, BLK], f8, tag="zn8t", name="zn8_2")
            zn8_3 = wrk.tile([128, KT, BLK], f8, tag="zn8t", name="zn8_3")

            norm_sq(0, 0, 512)          # DVE
            norm_ssq(0, 0, 512, ps0)    # PE
            norm_sq(0, 512, 1024)       # DVE
            rin00 = norm_act(0, 0, 512, ps0)        # ACT
            norm_ssq(0, 512, 1024, ps0)             # PE
            norm_sq(1, 0, BLK)                      # DVE
            rin01 = norm_act(0, 512, 1024, ps0)     # ACT
            norm_ssq(1, 0, BLK, ps1)                # PE
            norm_zn(0, 0, 512, znb0, rin00)         # DVE
            rin1 = norm_act(1, 0, BLK, ps1)         # ACT
            nc.gpsimd.dma_start(out=zn8_0[:, :, 0:512], in_=znb0[:, :, 0:512])
            norm_zn(0, 512, 1024, znb0, rin01)      # DVE
            nc.gpsimd.dma_start(out=zn8_0[:, :, 512:1024],
                                in_=znb0[:, :, 512:1024])
            norm_sq(2, 0, BLK)                      # DVE
            norm_ssq(2, 0, BLK, ps2)                # PE
            gram_exp(zn8_0, 0, 512, 0)              # PE + ACT (exps c0)
            rin2 = norm_act(2, 0, BLK, ps2)         # ACT
            norm_zn(1, 0, BLK, znb1, rin1)          # DVE
            nc.gpsimd.dma_start(out=zn8_1[:], in_=znb1[:])
            norm_sq(3, 0, BLK)                      # DVE
            norm_ssq(3, 0, BLK, ps3)                # PE
            gram_exp(zn8_0, 512, 1024, 1)           # PE + ACT (exps c1)
            rin3 = norm_act(3, 0, BLK, ps3)         # ACT
            norm_zn(2, 0, BLK, znb2, rin2)          # DVE
            nc.gpsimd.dma_start(out=zn8_2[:], in_=znb2[:])
            norm_zn(3, 0, BLK, znb3, rin3)          # DVE
            nc.gpsimd.dma_start(out=zn8_3[:], in_=znb3[:])

            gram_exp(zn8_1, 0, 1024, 2)             # PE + ACT (exps b1)

            # diag: exact self-dot of the quantized own columns (fp8)
            prodd = wrk.tile([128, KT, 512], bf16, tag="prodd", name="prodd")
            nc.vector.tensor_tensor(prodd[:], zn8_0[:, :, 0:512],
                                    zn8_0[:, :, 0:512], A.mult)
            dacc1 = wrk.tile([128, 2, 512], f32, tag="dacc1", name="dacc1")
            nc.vector.tensor_tensor(dacc1[:], prodd[:, 0:2, :], prodd[:, 2:4, :],
                                    A.add)
            dacc2 = wrk.tile([128, 512], f32, tag="dacc2", name="dacc2")
            nc.vector.tensor_tensor(dacc2[:], dacc1[:, 0, :], dacc1[:, 1, :],
                                    A.add)
            # positives: pos_r = zn_r . zn_{r+2048} in bf16 (partner rows
            # 2048..2559 = block-2 cols 0:512, same partition alignment)
            prodp = wrk.tile([128, KT, 512], bf16, tag="prodp", name="prodp")
            nc.vector.tensor_tensor(prodp[:], znb0[:, :, 0:512],
                                    znb2[:, :, 0:512], A.mult)
            pacc1 = wrk.tile([128, 2, 512], f32, tag="pacc1", name="pacc1")
            nc.vector.tensor_tensor(pacc1[:], prodp[:, 0:2, :], prodp[:, 2:4, :],
                                    A.add)
            pacc2 = wrk.tile([128, 512], f32, tag="pacc2", name="pacc2")
            nc.vector.tensor_tensor(pacc2[:], pacc1[:, 0, :], pacc1[:, 1, :],
                                    A.add)

            # all-reduces AFTER the last cast so they never block the fp8
            # cast pipeline on the single Pool SWDGE queue
            dg_bc = sb.tile([128, 512], f32, tag="dgbc")
            nc.gpsimd.partition_all_reduce(dg_bc[:], dacc2[:], channels=128,
                                           reduce_op=bass_isa.ReduceOp.add)
            diag_row = sb.tile([1, 512], bf16, tag="diagrow")
            nc.vector.tensor_scalar_add(diag_row[:], dg_bc[0:1, :],
                                        -FP8_SCALE ** 2)
            pos_bc = sb.tile([128, 512], f32, tag="posbc")
            nc.gpsimd.partition_all_reduce(pos_bc[:], pacc2[:], channels=128,
                                           reduce_op=bass_isa.ReduceOp.add)
            pos_red = sb.tile([128, 1], f32, tag="posr")
            nc.vector.tensor_reduce(pos_red[:], pos_bc[:], AX.X, A.add)

            gram_exp(zn8_2, 0, 1024, 3)             # PE + ACT (exps b2)

            # diag -> partition layout via K=1 outer-product matmuls
            dt = psS.tile([128, BLK], f32, tag="ssq", name="dt")
            for m in range(4):
                nc.tensor.matmul(dt[:, m * 128:(m + 1) * 128],
                                 diag_row[0:1, m * 128:(m + 1) * 128],
                                 ones[0:1, 0:128], start=True, stop=True)
            diag_part = sb.tile([128, 4], f32, tag="diagp")
            for m in range(4):
                nc.vector.tensor_copy(diag_part[:, m:m + 1],
                                      dt[:, m * 128:m * 128 + 1])
            dexp = sb.tile([128, 4], f32, tag="dexp")
            nc.scalar.activation(dexp[:], diag_part[:], F.Exp, scale=SC_DR,
                                 bias=bias10[:])

            gram_exp(zn8_3, 0, 1024, 4)             # PE + ACT (exps b3)

            # ---- finale: partial = sum_r ln(Z_r) - 10 * sum_r pos_r ----
            zsum = sb.tile([128, 4], f32, tag="zsum")
            for m in range(4):
                nc.vector.tensor_reduce(zsum[:, m:m + 1], rowp[:, m, :],
                                        AX.X, A.add)
            zarg = sb.tile([128, 4], f32, tag="zarg")
            nc.vector.tensor_tensor(zarg[:], zsum[:], dexp[:], A.subtract)
            logz = sb.tile([128, 5], f32, tag="logz")
            nc.scalar.activation(logz[:, 0:4], zarg[:], F.Ln)
            nc.vector.tensor_scalar_mul(logz[:, 4:5], pos_red[:], POS_MUL)
            red1 = sb.tile([128, 1], f32, tag="red1")
            nc.vector.tensor_reduce(red1[:], logz[:], AX.X, A.add)
            fin = sb.tile([1, 1], f32, tag="fin")
            nc.gpsimd.tensor_reduce(fin[:], red1[:], AX.C, A.add)
            nc.sync.dma_start(out=out, in_=fin[:])

    _patch_act_tables(nc, mybir)
    nc.compile()
    return nc


def _get_nc():
    if "nc" not in _CACHE:
        _CACHE["nc"] = _build()
    return _CACHE["nc"]


def _in_maps(z_i, z_j):
    import ml_dtypes

    zf = np.concatenate(
        [np.asarray(z_i, np.float32), np.asarray(z_j, np.float32)], axis=0)
    zb = zf.astype(ml_dtypes.bfloat16)
    maps = []
    for c in range(NCORES):
        zr = np.roll(zb, -c * RPC, axis=0)
        maps.append({"zt": np.ascontiguousarray(zr.T)})
    return maps


def _run(z_i, z_j, trace=False):
    from concourse.bass_utils import run_bass_kernel_spmd

    nc = _get_nc()
    return run_bass_kernel_spmd(nc, _in_maps(z_i, z_j), list(range(NCORES)),
                                trace=trace)


def kernel(z_i, z_j):
    res = _run(z_i, z_j, trace=False)
    total = sum(float(r["out"][0, 0]) for r in res.results)
    return np.float32(total / N2)


# revision 12
# speedup vs baseline: 1.1718x; 1.0197x over previous
"""NT-Xent (SimCLR contrastive) loss on Trainium2, sharded across 8 NeuronCores.

Sharding: each core computes a [512, 4096] row-slice of the similarity matrix.
The host ships each core a ROTATED copy of z (rows rolled by -512*core) cast to
bf16 - pure indexing/dtype prep, no host arithmetic. Rotation makes the device
program core-uniform: own rows are always rotated rows 0..511, the positive
partners always rotated rows 2048..2559. Row-sums of softmax are invariant to
the column permutation the rotation induces.

Device pipeline (per core, SPMD, pipelined per 1024-column block; block 0 is
processed in two 512-wide chunks to shorten the pipeline fill):
  - column-major z^T tiles materialized by XBAR transpose-DMAs straight from
    the row-major DRAM tensor (zbT[p,k,c] = z[c, 128k+p])
  - column norms: squares (DVE bf16 2x) pair-added so the ones-matmul
    contraction is K=256 instead of K=512
  - rinv16 = exp(-0.5*ln(ssq) + ln(16*1.045)) (one shared ACT table set)
  - zn = z * rinv16 in bf16 (DVE 2x), cast to fp8e4 by SWDGE cast-DMA
    (off-engine; 1.045 pre-scale compensates HW truncate-toward-zero)
  - Gram slice: fp8 DoubleRow matmuls, exp row-sums fused into ScalarE's
    activation accumulator
  - diagonal: recomputed exactly from the quantized fp8 values (elementwise
    square + gpsimd partition_all_reduce), subtracted before the final ln
  - positives: bf16 elementwise prod + partition_all_reduce + free-axis reduce
"""

import numpy as np

B = 2048
D = 512
N2 = 2 * B              # 4096 total rows
NCORES = 8
RPC = N2 // NCORES      # 512 rows per core
KT = D // 128           # 4 contraction tiles
BLK = 1024              # column-block size
NBLK = N2 // BLK        # 4 blocks
TEMP = 0.1
SCALE = 1.0 / TEMP      # 10.0
FP8_SCALE = 16.0        # zn stored as fp8(zn*16.72); effective x16 after trunc
TRUNC_COMP = 1.045      # SWDGE cast truncates toward zero; pre-scale by ~half
S16 = FP8_SCALE * TRUNC_COMP
LN_S16 = float(np.log(S16))
SC_DR = SCALE / (FP8_SCALE ** 2)        # exp scale for fp8 Gram psum
POS_MUL = -SCALE / (S16 * S16) / 128.0  # pos prods carry S16^2

_CACHE = {}


def _patch_act_tables(nc, mybir):
    """Make Ln and Exp resolve to the shared natural_log_exp_and_others set
    so the compiler emits one ACT table load instead of thrashing."""
    from concourse import hw_specs

    tables = hw_specs.get_activation_tables(nc.m.arch)
    keep = "natural_log_exp_and_others"
    if keep not in tables:
        return
    F = mybir.ActivationFunctionType
    if F.Exp not in tables[keep] or F.Ln not in tables[keep]:
        return
    for name, fns in tables.items():
        if name != keep:
            fns.discard(F.Exp)
            fns.discard(F.Ln)


def _build():
    from concourse import bass, bacc, tile, mybir, bass_isa

    nc = bacc.Bacc("TRN2", target_bir_lowering=False, debug=False,
                   num_devices=NCORES)
    bf16 = mybir.dt.bfloat16
    f32 = mybir.dt.float32
    f8 = mybir.dt.float8e4
    F = mybir.ActivationFunctionType
    A = mybir.AluOpType
    AX = mybir.AxisListType
    DR = mybir.MatmulPerfMode.DoubleRow
    PSUM = bass.MemorySpace.PSUM

    zt = nc.dram_tensor("zt", [D, N2], bf16, kind="ExternalInput").ap()
    out = nc.dram_tensor("out", [1, 1], f32, kind="ExternalOutput").ap()
    # [p, k, c] view: element (p, k, c) = zt[k*128 + p, c]
    zt_r = zt.rearrange("(k p) c -> p k c", k=KT)

    with tile.TileContext(nc) as tc:
        with (
            tc.tile_pool(name="sb", bufs=1) as sb,
            tc.tile_pool(name="wrk", bufs=2) as wrk,
            tc.tile_pool(name="psG", bufs=2, space=PSUM) as psG,
            tc.tile_pool(name="psS", bufs=2, space=PSUM) as psS,
        ):
            ones = sb.tile([128, 128], bf16, tag="ones")
            nc.vector.memset(ones[:], 1.0)
            bias16 = sb.tile([128, 1], f32, tag="b16")
            nc.vector.memset(bias16[:], LN_S16)
            bias10 = sb.tile([128, 1], f32, tag="b10")
            nc.vector.memset(bias10[:], SCALE)

            zbT = [sb.tile([128, KT, BLK], bf16, tag=f"zbT{b}", name=f"zbT{b}")
                   for b in range(NBLK)]
            znb0 = sb.tile([128, KT, BLK], bf16, tag="znb0")
            znb2 = sb.tile([128, KT, BLK], bf16, tag="znb2")
            zn8_0 = sb.tile([128, KT, BLK], f8, tag="zn8_0")
            rowp = sb.tile([128, 4, NBLK + 1], f32, tag="rowp")

            # PE p-state warm-up while the first DMAs stream in
            warm = psG.tile([128, BLK], f32, tag="gram", name="warm")
            for _ in range(8):
                nc.tensor.matmul(warm[:, 0:128], ones[:], ones[:],
                                 start=True, stop=True)

            # column-major tiles, spread over three DMA queues (SP, ACT
            # HWDGE, Pool SWDGE) so all input lands within ~5us
            nc.sync.dma_start(out=zbT[0][:, :, 0:512], in_=zt_r[:, :, 0:512])
            nc.scalar.dma_start(out=zbT[0][:, :, 512:1024],
                                in_=zt_r[:, :, 512:1024])
            nc.gpsimd.dma_start(out=zbT[3][:], in_=zt_r[:, :, 3 * BLK:4 * BLK])
            nc.sync.dma_start(out=zbT[1][:], in_=zt_r[:, :, BLK:2 * BLK])
            nc.scalar.dma_start(out=zbT[2][:], in_=zt_r[:, :, 2 * BLK:3 * BLK])

            # preload the Ln/Exp ACT table during the DMA window so the
            # 1.3us table load isn't on the first Ln's critical path
            junk1 = sb.tile([128, 1], f32, tag="junk1")
            nc.scalar.activation(junk1[:], bias10[:], F.Ln)

            # ---- split norm pipeline stages so each engine's in-order ----
            # ---- queue can be sequenced explicitly                    ----
            sqps = {}

            def norm_sq(b, lo, hi):
                """DVE: squares + pair-add over k (halves the ssq contraction)."""
                W = hi - lo
                sq = wrk.tile([128, KT, W], bf16, tag="sq", name="sq")
                nc.vector.tensor_tensor(sq[:], zbT[b][:, :, lo:hi],
                                        zbT[b][:, :, lo:hi], A.mult)
                sqp = wrk.tile([128, 2, W], bf16, tag="sqp", name="sqp")
                nc.vector.tensor_tensor(sqp[:], sq[:, 0:2, :], sq[:, 2:4, :],
                                        A.add)
                sqps[(b, lo)] = sqp

            def norm_ssq(b, lo, hi, pssq):
                """PE: ones-matmul partition reduction, K=2x128."""
                W = hi - lo
                sqp = sqps[(b, lo)]
                for g in range(2):
                    for j in range(W // 512):
                        nc.tensor.matmul(
                            pssq[:, lo + j * 512:lo + (j + 1) * 512],
                            ones[:], sqp[:, g, j * 512:(j + 1) * 512],
                            start=(g == 0), stop=(g == 1))

            def norm_act(b, lo, hi, pssq):
                """ACT: rinv16 = exp(-0.5*ln(ssq) + ln(16*1.045))."""
                W = hi - lo
                lns = wrk.tile([128, W], f32, tag="lns", name="lns")
                nc.scalar.activation(lns[:], pssq[:, lo:hi], F.Ln)
                rin = wrk.tile([128, W], bf16, tag="rin", name="rin")
                nc.scalar.activation(rin[:], lns[:], F.Exp, scale=-0.5,
                                     bias=bias16[:])
                return rin

            def norm_zn(b, lo, hi, zn_dst, rin):
                """DVE: zn = z * rinv16 (bf16, 2x mode)."""
                for k in range(KT):
                    nc.vector.tensor_tensor(zn_dst[:, k, lo:hi],
                                            zbT[b][:, k, lo:hi], rin[:], A.mult)

            def gram_exp(rhs8, lo, hi, col):
                """PE DR Gram of own rows x cols [lo,hi); ACT exp row-sums."""
                W = hi - lo
                for m in range(4):
                    pm = psG.tile([128, BLK], f32, tag="gram", name="pm")
                    for g in range(2):
                        for j in range(W // 512):
                            nc.tensor.matmul(
                                pm[:, j * 512:(j + 1) * 512],
                                zn8_0[:, 2 * g:2 * g + 2, m * 128:(m + 1) * 128],
                                rhs8[:, 2 * g:2 * g + 2, lo + j * 512:lo + (j + 1) * 512],
                                start=(g == 0), stop=(g == 1), perf_mode=DR)
                    scr = wrk.tile([128, W], bf16, tag="scr", name="scr")
                    nc.scalar.activation(scr[:], pm[:, 0:W], F.Exp, scale=SC_DR,
                                         accum_out=rowp[:, m, col:col + 1])

            # ---- pipeline: block-0 chain emitted dependency-first so the
            # ---- first exps start as early as possible; later blocks
            # ---- interleave; pos/diag via ones-matmul PE reductions
            ps0 = psS.tile([128, BLK], f32, tag="ssq", name="ps0")
            ps1 = psS.tile([128, BLK], f32, tag="ssq", name="ps1")
            ps2 = psS.tile([128, BLK], f32, tag="ssq", name="ps2")
            ps3 = psS.tile([128, BLK], f32, tag="ssq", name="ps3")
            znb1 = wrk.tile([128, KT, BLK], bf16, tag="znt", name="znb1")
            znb3 = wrk.tile([128, KT, BLK], bf16, tag="znt", name="znb3")
            zn8_1 = wrk.tile([128, KT, BLK], f8, tag="zn8t", name="zn8_1")
            zn8_2 = wrk.tile([128, KT, BLK], f8, tag="zn8t", name="zn8_2")
            zn8_3 = wrk.tile([128, KT, BLK], f8, tag="zn8t", name="zn8_3")

            norm_sq(0, 0, 512)
            norm_ssq(0, 0, 512, ps0)
            rin00 = norm_act(0, 0, 512, ps0)
            norm_zn(0, 0, 512, znb0, rin00)
            nc.gpsimd.dma_start(out=zn8_0[:, :, 0:512], in_=znb0[:, :, 0:512])

            norm_sq(0, 512, 1024)
            norm_ssq(0, 512, 1024, ps0)
            rin01 = norm_act(0, 512, 1024, ps0)
            norm_zn(0, 512, 1024, znb0, rin01)
            nc.gpsimd.dma_start(out=zn8_0[:, :, 512:1024],
                                in_=znb0[:, :, 512:1024])

            norm_sq(1, 0, BLK)
            norm_ssq(1, 0, BLK, ps1)
            rin1 = norm_act(1, 0, BLK, ps1)

            gram_exp(zn8_0, 0, 512, 0)              # first exps ASAP

            norm_zn(1, 0, BLK, znb1, rin1)
            nc.gpsimd.dma_start(out=zn8_1[:], in_=znb1[:])
            norm_sq(2, 0, BLK)
            norm_ssq(2, 0, BLK, ps2)
            rin2 = norm_act(2, 0, BLK, ps2)

            gram_exp(zn8_0, 512, 1024, 1)

            norm_zn(2, 0, BLK, znb2, rin2)
            nc.gpsimd.dma_start(out=zn8_2[:], in_=znb2[:])
            norm_sq(3, 0, BLK)
            norm_ssq(3, 0, BLK, ps3)
            rin3 = norm_act(3, 0, BLK, ps3)

            gram_exp(zn8_1, 0, 1024, 2)

            norm_zn(3, 0, BLK, znb3, rin3)
            nc.gpsimd.dma_start(out=zn8_3[:], in_=znb3[:])

            # diag: exact self-dot of the quantized own columns (fp8);
            # partition-reduce via K=128 ones-matmuls (exact f32 accum)
            prodd = wrk.tile([128, KT, 512], bf16, tag="prodd", name="prodd")
            nc.vector.tensor_tensor(prodd[:], zn8_0[:, :, 0:512],
                                    zn8_0[:, :, 0:512], A.mult)
            dg = psS.tile([128, BLK], f32, tag="ssq", name="dg")
            for k in range(KT):
                nc.tensor.matmul(dg[0:1, 0:512], ones[:, 0:1], prodd[:, k, :],
                                 start=(k == 0), stop=(k == KT - 1))
            diag_row = sb.tile([1, 512], bf16, tag="diagrow")
            nc.vector.tensor_scalar_add(diag_row[:], dg[0:1, 0:512],
                                        -FP8_SCALE ** 2)

            gram_exp(zn8_2, 0, 1024, 3)

            # positives: pos_r = zn_r . zn_{r+2048} in bf16 (partner rows
            # 2048..2559 = block-2 cols 0:512, same partition alignment)
            prodp = wrk.tile([128, KT, 512], bf16, tag="prodp", name="prodp")
            nc.vector.tensor_tensor(prodp[:], znb0[:, :, 0:512],
                                    znb2[:, :, 0:512], A.mult)
            pp = psS.tile([128, BLK], f32, tag="ssq", name="pp")
            for k in range(KT):
                nc.tensor.matmul(pp[:, 0:512], ones[:], prodp[:, k, :],
                                 start=(k == 0), stop=(k == KT - 1))
            pos_red = sb.tile([128, 1], f32, tag="posr")
            nc.vector.tensor_reduce(pos_red[:], pp[:, 0:512], AX.X, A.add)

            # diag -> partition layout via K=1 outer-product matmuls
            dt = psS.tile([128, BLK], f32, tag="ssq", name="dt")
            for m in range(4):
                nc.tensor.matmul(dt[:, m * 128:(m + 1) * 128],
                                 diag_row[0:1, m * 128:(m + 1) * 128],
                                 ones[0:1, 0:128], start=True, stop=True)
            diag_part = sb.tile([128, 4], f32, tag="diagp")
            for m in range(4):
                nc.vector.tensor_copy(diag_part[:, m:m + 1],
                                      dt[:, m * 128:m * 128 + 1])
            dexp = sb.tile([128, 4], f32, tag="dexp")
            nc.scalar.activation(dexp[:], diag_part[:], F.Exp, scale=SC_DR,
                                 bias=bias10[:])

            gram_exp(zn8_3, 0, 1024, 4)

            # ---- finale: partial = sum_r ln(Z_r) - 10 * sum_r pos_r ----
            zsum = sb.tile([128, 4], f32, tag="zsum")
            for m in range(4):
                nc.vector.tensor_reduce(zsum[:, m:m + 1], rowp[:, m, :],
                                        AX.X, A.add)
            zarg = sb.tile([128, 4], f32, tag="zarg")
            nc.vector.tensor_tensor(zarg[:], zsum[:], dexp[:], A.subtract)
            logz = sb.tile([128, 5], f32, tag="logz")
            nc.scalar.activation(logz[:, 0:4], zarg[:], F.Ln)
            nc.vector.tensor_scalar_mul(logz[:, 4:5], pos_red[:], POS_MUL)
            red1 = sb.tile([128, 1], f32, tag="red1")
            nc.vector.tensor_reduce(red1[:], logz[:], AX.X, A.add)
            fin = sb.tile([1, 1], f32, tag="fin")
            nc.gpsimd.tensor_reduce(fin[:], red1[:], AX.C, A.add)
            nc.sync.dma_start(out=out, in_=fin[:])

    _patch_act_tables(nc, mybir)
    nc.compile()
    return nc


def _get_nc():
    if "nc" not in _CACHE:
        _CACHE["nc"] = _build()
    return _CACHE["nc"]


def _in_maps(z_i, z_j):
    import ml_dtypes

    zf = np.concatenate(
        [np.asarray(z_i, np.float32), np.asarray(z_j, np.float32)], axis=0)
    zb = zf.astype(ml_dtypes.bfloat16)
    maps = []
    for c in range(NCORES):
        zr = np.roll(zb, -c * RPC, axis=0)
        maps.append({"zt": np.ascontiguousarray(zr.T)})
    return maps


def _run(z_i, z_j, trace=False):
    from concourse.bass_utils import run_bass_kernel_spmd

    nc = _get_nc()
    return run_bass_kernel_spmd(nc, _in_maps(z_i, z_j), list(range(NCORES)),
                                trace=trace)


def kernel(z_i, z_j):
    res = _run(z_i, z_j, trace=False)
    total = sum(float(r["out"][0, 0]) for r in res.results)
    return np.float32(total / N2)


# revision 13
# speedup vs baseline: 1.2168x; 1.0384x over previous
"""NT-Xent (SimCLR contrastive) loss on Trainium2, sharded across 8 NeuronCores.

Sharding: each core computes a [512, 4096] row-slice of the similarity matrix.
The host ships each core a ROTATED copy of z (rows rolled by -512*core) cast to
bf16 - pure indexing/dtype prep, no host arithmetic. Rotation makes the device
program core-uniform: own rows are always rotated rows 0..511, the positive
partners always rotated rows 2048..2559. Row-sums of softmax are invariant to
the column permutation the rotation induces.

Device pipeline (per core, SPMD, pipelined per 1024-column block; block 0 is
processed in two 512-wide chunks to shorten the pipeline fill):
  - column-major z^T tiles materialized by XBAR transpose-DMAs straight from
    the row-major DRAM tensor (zbT[p,k,c] = z[c, 128k+p])
  - column norms: squares (DVE bf16 2x) pair-added so the ones-matmul
    contraction is K=256 instead of K=512
  - rinv16 = exp(-0.5*ln(ssq) + ln(16*1.045)) (one shared ACT table set)
  - zn = z * rinv16 in bf16 (DVE 2x), cast to fp8e4 by SWDGE cast-DMA
    (off-engine; 1.045 pre-scale compensates HW truncate-toward-zero)
  - Gram slice: fp8 DoubleRow matmuls, exp row-sums fused into ScalarE's
    activation accumulator
  - diagonal: recomputed exactly from the quantized fp8 values (elementwise
    square + gpsimd partition_all_reduce), subtracted before the final ln
  - positives: bf16 elementwise prod + partition_all_reduce + free-axis reduce
"""

import numpy as np

B = 2048
D = 512
N2 = 2 * B              # 4096 total rows
NCORES = 8
RPC = N2 // NCORES      # 512 rows per core
KT = D // 128           # 4 contraction tiles
BLK = 1024              # column-block size
NBLK = N2 // BLK        # 4 blocks
TEMP = 0.1
SCALE = 1.0 / TEMP      # 10.0
FP8_SCALE = 16.0        # zn stored as fp8(zn*16.72); effective x16 after trunc
TRUNC_COMP = 1.045      # SWDGE cast truncates toward zero; pre-scale by ~half
S16 = FP8_SCALE * TRUNC_COMP
LN_S16 = float(np.log(S16))
SC_DR = SCALE / (FP8_SCALE ** 2)        # exp scale for fp8 Gram psum
POS_MUL = -SCALE / (S16 * S16) / 128.0  # pos prods carry S16^2

_CACHE = {}


def _patch_act_tables(nc, mybir):
    """Make Ln and Exp resolve to the shared natural_log_exp_and_others set
    so the compiler emits one ACT table load instead of thrashing."""
    from concourse import hw_specs

    tables = hw_specs.get_activation_tables(nc.m.arch)
    keep = "natural_log_exp_and_others"
    if keep not in tables:
        return
    F = mybir.ActivationFunctionType
    if F.Exp not in tables[keep] or F.Ln not in tables[keep]:
        return
    for name, fns in tables.items():
        if name != keep:
            fns.discard(F.Exp)
            fns.discard(F.Ln)


def _build():
    from concourse import bass, bacc, tile, mybir, bass_isa

    nc = bacc.Bacc("TRN2", target_bir_lowering=False, debug=False,
                   num_devices=NCORES)
    bf16 = mybir.dt.bfloat16
    f32 = mybir.dt.float32
    f8 = mybir.dt.float8e4
    F = mybir.ActivationFunctionType
    A = mybir.AluOpType
    AX = mybir.AxisListType
    DR = mybir.MatmulPerfMode.DoubleRow
    PSUM = bass.MemorySpace.PSUM

    # block-major column-major layout: ztb[b, p, k, c] = z_rot[b*1024+c, k*128+p]
    # so each block is one DMA with 8KB-contiguous per-partition descriptors
    ztb = nc.dram_tensor("ztb", [NBLK, 128, KT, BLK], bf16,
                         kind="ExternalInput").ap()
    out = nc.dram_tensor("out", [1, 1], f32, kind="ExternalOutput").ap()

    with tile.TileContext(nc) as tc:
        with (
            tc.tile_pool(name="sb", bufs=1) as sb,
            tc.tile_pool(name="wrk", bufs=2) as wrk,
            tc.tile_pool(name="psG", bufs=2, space=PSUM) as psG,
            tc.tile_pool(name="psS", bufs=2, space=PSUM) as psS,
        ):
            ones = sb.tile([128, 128], bf16, tag="ones")
            nc.vector.memset(ones[:], 1.0)
            bias16 = sb.tile([128, 1], f32, tag="b16")
            nc.vector.memset(bias16[:], LN_S16)
            bias10 = sb.tile([128, 1], f32, tag="b10")
            nc.vector.memset(bias10[:], SCALE)

            zbT = [sb.tile([128, KT, BLK], bf16, tag=f"zbT{b}", name=f"zbT{b}")
                   for b in range(NBLK)]
            znb0 = sb.tile([128, KT, BLK], bf16, tag="znb0")
            znb2 = sb.tile([128, KT, BLK], bf16, tag="znb2")
            zn8_0 = sb.tile([128, KT, BLK], f8, tag="zn8_0")
            rowp = sb.tile([128, 4, NBLK + 1], f32, tag="rowp")

            # PE p-state warm-up while the first DMAs stream in
            warm = psG.tile([128, BLK], f32, tag="gram", name="warm")
            for _ in range(8):
                nc.tensor.matmul(warm[:, 0:128], ones[:], ones[:],
                                 start=True, stop=True)

            # column-major tiles, spread over three DMA queues (SP, ACT
            # HWDGE, Pool SWDGE) so all input lands within ~5us
            nc.sync.dma_start(out=zbT[0][:], in_=ztb[0])
            nc.scalar.dma_start(out=zbT[1][:], in_=ztb[1])
            nc.gpsimd.dma_start(out=zbT[3][:], in_=ztb[3])
            nc.sync.dma_start(out=zbT[2][:], in_=ztb[2])

            # preload the Ln/Exp ACT table during the DMA window so the
            # 1.3us table load isn't on the first Ln's critical path
            junk1 = sb.tile([128, 1], f32, tag="junk1")
            nc.scalar.activation(junk1[:], bias10[:], F.Ln)

            # ---- split norm pipeline stages so each engine's in-order ----
            # ---- queue can be sequenced explicitly                    ----
            sqps = {}

            def norm_sq(b, lo, hi):
                """DVE: squares + pair-add over k (halves the ssq contraction)."""
                W = hi - lo
                sq = wrk.tile([128, KT, W], bf16, tag="sq", name="sq")
                nc.vector.tensor_tensor(sq[:], zbT[b][:, :, lo:hi],
                                        zbT[b][:, :, lo:hi], A.mult)
                sqp = wrk.tile([128, 2, W], bf16, tag="sqp", name="sqp")
                nc.vector.tensor_tensor(sqp[:], sq[:, 0:2, :], sq[:, 2:4, :],
                                        A.add)
                sqps[(b, lo)] = sqp

            def norm_ssq(b, lo, hi, pssq):
                """PE: ones-matmul partition reduction, K=2x128."""
                W = hi - lo
                sqp = sqps[(b, lo)]
                for g in range(2):
                    for j in range(W // 512):
                        nc.tensor.matmul(
                            pssq[:, lo + j * 512:lo + (j + 1) * 512],
                            ones[:], sqp[:, g, j * 512:(j + 1) * 512],
                            start=(g == 0), stop=(g == 1))

            def norm_act(b, lo, hi, pssq):
                """ACT: rinv16 = exp(-0.5*ln(ssq) + ln(16*1.045))."""
                W = hi - lo
                lns = wrk.tile([128, W], f32, tag="lns", name="lns")
                nc.scalar.activation(lns[:], pssq[:, lo:hi], F.Ln)
                rin = wrk.tile([128, W], bf16, tag="rin", name="rin")
                nc.scalar.activation(rin[:], lns[:], F.Exp, scale=-0.5,
                                     bias=bias16[:])
                return rin

            def norm_zn(b, lo, hi, zn_dst, rin):
                """DVE: zn = z * rinv16 (bf16, 2x mode)."""
                for k in range(KT):
                    nc.vector.tensor_tensor(zn_dst[:, k, lo:hi],
                                            zbT[b][:, k, lo:hi], rin[:], A.mult)

            def gram_exp(rhs8, lo, hi, col):
                """PE DR Gram of own rows x cols [lo,hi); ACT exp row-sums."""
                W = hi - lo
                for m in range(4):
                    pm = psG.tile([128, BLK], f32, tag="gram", name="pm")
                    for g in range(2):
                        for j in range(W // 512):
                            nc.tensor.matmul(
                                pm[:, j * 512:(j + 1) * 512],
                                zn8_0[:, 2 * g:2 * g + 2, m * 128:(m + 1) * 128],
                                rhs8[:, 2 * g:2 * g + 2, lo + j * 512:lo + (j + 1) * 512],
                                start=(g == 0), stop=(g == 1), perf_mode=DR)
                    scr = wrk.tile([128, W], bf16, tag="scr", name="scr")
                    nc.scalar.activation(scr[:], pm[:, 0:W], F.Exp, scale=SC_DR,
                                         accum_out=rowp[:, m, col:col + 1])

            # ---- pipeline: block-0 chain emitted dependency-first so the
            # ---- first exps start as early as possible; later blocks
            # ---- interleave; pos/diag via ones-matmul PE reductions
            ps0 = psS.tile([128, BLK], f32, tag="ssq", name="ps0")
            ps1 = psS.tile([128, BLK], f32, tag="ssq", name="ps1")
            ps2 = psS.tile([128, BLK], f32, tag="ssq", name="ps2")
            ps3 = psS.tile([128, BLK], f32, tag="ssq", name="ps3")
            znb1 = wrk.tile([128, KT, BLK], bf16, tag="znt", name="znb1")
            znb3 = wrk.tile([128, KT, BLK], bf16, tag="znt", name="znb3")
            zn8_1 = wrk.tile([128, KT, BLK], f8, tag="zn8t", name="zn8_1")
            zn8_2 = wrk.tile([128, KT, BLK], f8, tag="zn8t", name="zn8_2")
            zn8_3 = wrk.tile([128, KT, BLK], f8, tag="zn8t", name="zn8_3")

            norm_sq(0, 0, 512)
            norm_ssq(0, 0, 512, ps0)
            rin00 = norm_act(0, 0, 512, ps0)
            norm_zn(0, 0, 512, znb0, rin00)
            nc.gpsimd.dma_start(out=zn8_0[:, :, 0:512], in_=znb0[:, :, 0:512])

            norm_sq(0, 512, 1024)
            norm_ssq(0, 512, 1024, ps0)
            rin01 = norm_act(0, 512, 1024, ps0)
            norm_zn(0, 512, 1024, znb0, rin01)
            nc.gpsimd.dma_start(out=zn8_0[:, :, 512:1024],
                                in_=znb0[:, :, 512:1024])

            norm_sq(1, 0, BLK)
            norm_ssq(1, 0, BLK, ps1)
            rin1 = norm_act(1, 0, BLK, ps1)

            gram_exp(zn8_0, 0, 512, 0)              # first exps ASAP

            norm_zn(1, 0, BLK, znb1, rin1)
            nc.gpsimd.dma_start(out=zn8_1[:], in_=znb1[:])
            norm_sq(2, 0, BLK)
            norm_ssq(2, 0, BLK, ps2)
            rin2 = norm_act(2, 0, BLK, ps2)

            gram_exp(zn8_0, 512, 1024, 1)

            norm_zn(2, 0, BLK, znb2, rin2)
            nc.gpsimd.dma_start(out=zn8_2[:], in_=znb2[:])
            norm_sq(3, 0, BLK)
            norm_ssq(3, 0, BLK, ps3)
            rin3 = norm_act(3, 0, BLK, ps3)

            gram_exp(zn8_1, 0, 1024, 2)

            norm_zn(3, 0, BLK, znb3, rin3)
            nc.gpsimd.dma_start(out=zn8_3[:], in_=znb3[:])

            # diag: exact self-dot of the quantized own columns (fp8);
            # partition-reduce via K=128 ones-matmuls (exact f32 accum)
            prodd = wrk.tile([128, KT, 512], bf16, tag="prodd", name="prodd")
            nc.vector.tensor_tensor(prodd[:], zn8_0[:, :, 0:512],
                                    zn8_0[:, :, 0:512], A.mult)
            dg = psS.tile([128, BLK], f32, tag="ssq", name="dg")
            for k in range(KT):
                nc.tensor.matmul(dg[0:1, 0:512], ones[:, 0:1], prodd[:, k, :],
                                 start=(k == 0), stop=(k == KT - 1))
            diag_row = sb.tile([1, 512], bf16, tag="diagrow")
            nc.vector.tensor_scalar_add(diag_row[:], dg[0:1, 0:512],
                                        -FP8_SCALE ** 2)

            gram_exp(zn8_2, 0, 1024, 3)

            # positives: pos_r = zn_r . zn_{r+2048} in bf16 (partner rows
            # 2048..2559 = block-2 cols 0:512, same partition alignment)
            prodp = wrk.tile([128, KT, 512], bf16, tag="prodp", name="prodp")
            nc.vector.tensor_tensor(prodp[:], znb0[:, :, 0:512],
                                    znb2[:, :, 0:512], A.mult)
            pp = psS.tile([128, BLK], f32, tag="ssq", name="pp")
            for k in range(KT):
                nc.tensor.matmul(pp[:, 0:512], ones[:], prodp[:, k, :],
                                 start=(k == 0), stop=(k == KT - 1))
            pos_red = sb.tile([128, 1], f32, tag="posr")
            nc.vector.tensor_reduce(pos_red[:], pp[:, 0:512], AX.X, A.add)

            # diag -> partition layout via K=1 outer-product matmuls
            dt = psS.tile([128, BLK], f32, tag="ssq", name="dt")
            for m in range(4):
                nc.tensor.matmul(dt[:, m * 128:(m + 1) * 128],
                                 diag_row[0:1, m * 128:(m + 1) * 128],
                                 ones[0:1, 0:128], start=True, stop=True)
            diag_part = sb.tile([128, 4], f32, tag="diagp")
            for m in range(4):
                nc.vector.tensor_copy(diag_part[:, m:m + 1],
                                      dt[:, m * 128:m * 128 + 1])
            dexp = sb.tile([128, 4], f32, tag="dexp")
            nc.scalar.activation(dexp[:], diag_part[:], F.Exp, scale=SC_DR,
                                 bias=bias10[:])

            gram_exp(zn8_3, 0, 1024, 4)

            # ---- finale: partial = sum_r ln(Z_r) - 10 * sum_r pos_r ----
            zsum = sb.tile([128, 4], f32, tag="zsum")
            for m in range(4):
                nc.vector.tensor_reduce(zsum[:, m:m + 1], rowp[:, m, :],
                                        AX.X, A.add)
            zarg = sb.tile([128, 4], f32, tag="zarg")
            nc.vector.tensor_tensor(zarg[:], zsum[:], dexp[:], A.subtract)
            logz = sb.tile([128, 5], f32, tag="logz")
            nc.scalar.activation(logz[:, 0:4], zarg[:], F.Ln)
            nc.vector.tensor_scalar_mul(logz[:, 4:5], pos_red[:], POS_MUL)
            red1 = sb.tile([128, 1], f32, tag="red1")
            nc.vector.tensor_reduce(red1[:], logz[:], AX.X, A.add)
            fin = sb.tile([1, 1], f32, tag="fin")
            nc.gpsimd.tensor_reduce(fin[:], red1[:], AX.C, A.add)
            nc.sync.dma_start(out=out, in_=fin[:])

    _patch_act_tables(nc, mybir)
    nc.compile()
    return nc


def _get_nc():
    if "nc" not in _CACHE:
        _CACHE["nc"] = _build()
    return _CACHE["nc"]


def _in_maps(z_i, z_j):
    import ml_dtypes

    zf = np.concatenate(
        [np.asarray(z_i, np.float32), np.asarray(z_j, np.float32)], axis=0)
    zb = zf.astype(ml_dtypes.bfloat16)
    maps = []
    for c in range(NCORES):
        zr = np.roll(zb, -c * RPC, axis=0)
        # ztb[b, p, k, c] = zr[b*1024 + c, k*128 + p]
        ztb = np.transpose(zr.reshape(NBLK, BLK, KT, 128), (0, 3, 2, 1))
        maps.append({"ztb": np.ascontiguousarray(ztb)})
    return maps


def _run(z_i, z_j, trace=False):
    from concourse.bass_utils import run_bass_kernel_spmd

    nc = _get_nc()
    return run_bass_kernel_spmd(nc, _in_maps(z_i, z_j), list(range(NCORES)),
                                trace=trace)


def kernel(z_i, z_j):
    res = _run(z_i, z_j, trace=False)
    total = sum(float(r["out"][0, 0]) for r in res.results)
    return np.float32(total / N2)
